# revision 15
# baseline (speedup 1.0000x reference)
"""Trainium2 Bass kernel for nn_MultiHeadAttention_59158879535767 (v3).

Reference semantics (B=4, S=2048, D=1024, H=16, DK=64):
  Q = q @ w_q.T + b_q  (same for K, V), reshaped (B,S,DK,H);
  score contracts over the HEAD axis per token: score[t] = Q_t @ K_t.T / 8
  (64x64 per token), softmax over last axis, attn[t] = score @ V_t -> (64,16),
  flattened, then @ w_o.T + b_o.

Everything is per-token => data-parallel over the 8192 tokens across 8 cores.
Per core T=1024 tokens in NQ=4 quarters of T4=256, in 32-token batches.

v3 (from v2b @ 470us, baseline 698us): one fully interleaved emission stream.
The PE must never idle >~3us or the HAM clock gate drops it from 2.4 to
1.2 GHz (measured: whole oproj phases ran cold).  Per quarter we emit
[3 proj j-chains + 1 attention batch] x 8 so that:
  * the PE queue always has dense matmul work while ACT runs exp and the
    strided evacs trail on DVE/ACT without blocking consumers queued behind
    a wholesale evac block (queues are strict FIFO per engine);
  * attention batches are depth-2 software-pipelined (S2(b-1) after S1(b))
    so the exp latency hides under the next batch's transposes;
  * oproj(pair0) units interleave into quarter 3, oproj(pair1) drains at
    the end (dense, so it stays warm).
Engine split: DVE = Q evacs (strided) + pack copies + normalize;
ACT = K/V evacs + exp + oproj evac.  Fused Q|K pack -> one [128,128]
transpose per 4 tokens; normalize straight from PSUM; initial weight DMAs
spread across engine queues with x(0) first.
"""
import numpy as np
import ml_dtypes

B, S, D, H, DK = 4, 2048, 1024, 16, 64
NCORE = 8
T = (B * S) // NCORE          # 1024 tokens per core
NQ = 4
T4 = T // NQ                  # 256 tokens per quarter
NB = T4 // 32                 # 8 batches of 32 tokens per quarter
TP2 = 256                     # tokens per parity per 2-quarter pair

bf16 = ml_dtypes.bfloat16

_NC_CACHE = {}


def build_nc():
    import concourse.bacc as bacc
    import concourse.mybir as mybir
    import concourse.tile as tile
    from concourse.masks import make_identity

    nc = bacc.Bacc()
    dt = mybir.dt
    f32, b16 = dt.float32, dt.bfloat16
    AF = mybir.ActivationFunctionType

    # ---- DRAM I/O ----
    xq_d = nc.dram_tensor("xq", [8, 128, T], b16, kind="ExternalInput")
    xk_d = nc.dram_tensor("xk", [8, 128, T], b16, kind="ExternalInput")
    xv_d = nc.dram_tensor("xv", [8, 128, T], b16, kind="ExternalInput")
    wq_d = nc.dram_tensor("wq", [8, 128, 1024], b16, kind="ExternalInput")
    wk_d = nc.dram_tensor("wk", [8, 128, 1024], b16, kind="ExternalInput")
    wv_d = nc.dram_tensor("wv", [8, 128, 1024], b16, kind="ExternalInput")
    wo_d = nc.dram_tensor("wo", [128, 16, 1024], b16, kind="ExternalInput")
    bq_d = nc.dram_tensor("bq", [8, 128], f32, kind="ExternalInput")
    bk_d = nc.dram_tensor("bk", [8, 128], f32, kind="ExternalInput")
    bv_d = nc.dram_tensor("bv", [8, 128], f32, kind="ExternalInput")
    bo_d = nc.dram_tensor("bo", [8, 128], f32, kind="ExternalInput")
    # out token map: t = qp*512 + qh*256 + 2*tp + par, D = m*128 + dm
    out_d = nc.dram_tensor("out", [2, 2, 8, 128, TP2], f32, kind="ExternalOutput")

    with tile.TileContext(nc) as tc:
        with (
            tc.tile_pool(name="const", bufs=1) as const,
            tc.tile_pool(name="xin", bufs=2) as xin,
            tc.tile_pool(name="work", bufs=2) as work,
            tc.tile_pool(name="qf", bufs=1) as qfp,
            tc.tile_pool(name="epool", bufs=2) as epool,
            tc.tile_pool(name="zpool", bufs=4) as zpool,
            tc.tile_pool(name="outp", bufs=4) as outp,
            tc.tile_pool(name="psp", bufs=3, space="PSUM") as psp,
            tc.tile_pool(name="psa", bufs=5, space="PSUM") as psa,
        ):
            # ---- persistent SBUF ----
            wq_s = const.tile([128, 8, 1024], b16, tag="wq")
            wk_s = const.tile([128, 8, 1024], b16, tag="wk")
            wv_s = const.tile([128, 8, 1024], b16, tag="wv")
            wo_s = const.tile([128, 16, 1024], b16, tag="wo")
            bq_s = const.tile([128, 8], f32, tag="bq")
            bk_s = const.tile([128, 8], f32, tag="bk")
            bv_s = const.tile([128, 8], f32, tag="bv")
            bo_s = const.tile([128, 8], f32, tag="bo")
            ident = const.tile([128, 128], b16, tag="ident")

            # per-quarter staging (x2: attn(q-1) overlaps proj(q))
            # qk: rows 0:64 Q^T [d, t, s], rows 64:128 K^T [e, t, s]
            qk_s = const.tile([128, 2, T4, 32], b16, tag="qk")
            # stg: rows 0:64 = K even-h (-> up-shift), 64:128 = Q odd-h (-> down)
            stg_s = const.tile([128, 2, T4, 8], b16, tag="stg")
            vt_s = const.tile([128, 2, 17, T4], b16, tag="vt")
            osv_s = const.tile([128, 2, 8, T4], b16, tag="osv")
            a_nm = const.tile([128, 2, TP2, 16], b16, tag="anorm")

            # x(0) first so proj(0) can start ASAP; weights spread across
            # engine DMA queues so they load in parallel with x and each other
            def load_x(qq):
                tsl = slice(qq * T4, (qq + 1) * T4)
                xt = []
                for nm, xd in (("xq", xq_d), ("xk", xk_d), ("xv", xv_d)):
                    t_ = xin.tile([128, 8, T4], b16, tag=nm)
                    nc.sync.dma_start(out=t_[:], in_=xd[:, :, tsl].rearrange("ko p t -> p ko t"))
                    xt.append(t_)
                return xt

            x_cur = load_x(0)
            for bd, bsb in ((bq_d, bq_s), (bk_d, bk_s), (bv_d, bv_s), (bo_d, bo_s)):
                nc.sync.dma_start(out=bsb[:], in_=bd.rearrange("j p -> p j"))
            # weight loads split across both hwdge queues, x(0) already ahead
            nc.scalar.dma_start(out=wq_s[:], in_=wq_d.rearrange("ko p m -> p ko m"))
            nc.scalar.dma_start(out=wk_s[:], in_=wk_d.rearrange("ko p m -> p ko m"))
            nc.scalar.dma_start(out=wv_s[:], in_=wv_d.rearrange("ko p m -> p ko m"))
            nc.sync.dma_start(out=wo_s[:], in_=wo_d[:])
            make_identity(nc, ident)
            nc.any.memset(qk_s[:, :, :, 16:32], 0.0)
            nc.any.memset(vt_s[0:64, :, 16, :], 1.0)

            W = {"q": (wq_s, bq_s), "k": (wk_s, bk_s), "v": (wv_s, bv_s)}

            # per-quarter contiguous projection staging (pre-scatter)
            qf_cur = {}

            def chain(qq, kind, j, x_t):
                """One projection j-chain: 8 accumulating MMs + evac.

                PSUM evac is ALWAYS contiguous on ACT (357ns measured; a
                strided ACT/DVE evac costs 1.1-1.6us).  Q/K land in a flat
                [128, 8, T4] staging tile; one bulk DVE SBUF->SBUF scatter
                per projection half then builds the t-major pack layout
                (strided SBUF->SBUF on DVE is cheap)."""
                qi = qq % 2
                x = x_t["qkv".index(kind)]
                w_s, b_s = W[kind]
                pj = psp.tile([128, 512], f32, tag="ps", name="pj")[:, :T4]
                for ko in range(8):
                    nc.tensor.matmul(
                        pj, w_s[:, ko, j * 128:(j + 1) * 128], x[:, ko, :],
                        start=(ko == 0), stop=(ko == 7))
                if kind in ("q", "k"):
                    if j == 0:
                        qf_cur[kind] = qfp.tile(
                            [128, 8, T4], b16, tag="qf" + kind, name="qf" + kind)
                    qf = qf_cur[kind]
                    nc.scalar.activation(
                        qf[:, j, :], pj[:, :],
                        AF.Identity, bias=b_s[:, j:j + 1], scale=1.0)
                    if j == 7:
                        src_lo = qf[0:64, :, :].rearrange("p j t -> p t j")
                        src_hi = qf[64:128, :, :].rearrange("p j t -> p t j")
                        if kind == "q":
                            # even h -> direct slots 0:8; odd h -> stg (down)
                            nc.vector.tensor_copy(qk_s[0:64, qi, :, 0:8], src_lo)
                            nc.vector.tensor_copy(stg_s[64:128, qi, :, :], src_hi)
                            for cc in range(4):
                                csl = slice(64 * cc, 64 * cc + 64)
                                psh = psp.tile([128, 512], f32, tag="ps",
                                               name="pshq")
                                nc.tensor.matmul(
                                    psh[0:64, :], ident[64:128, 64:128],
                                    stg_s[64:128, qi, csl, :],
                                    start=True, stop=True,
                                    tile_position=(64, 0))
                                nc.scalar.activation(
                                    qk_s[0:64, qi, csl, 8:16], psh[0:64, :],
                                    AF.Identity, scale=1.0)
                        else:
                            # even h -> stg (up); odd h -> direct slots 8:16
                            nc.vector.tensor_copy(stg_s[0:64, qi, :, :], src_lo)
                            nc.vector.tensor_copy(qk_s[64:128, qi, :, 8:16], src_hi)
                            for cc in range(4):
                                csl = slice(64 * cc, 64 * cc + 64)
                                psh = psp.tile([128, 512], f32, tag="ps",
                                               name="pshk")
                                nc.tensor.matmul(
                                    psh[64:128, :], ident[0:64, 0:64],
                                    stg_s[0:64, qi, csl, :],
                                    start=True, stop=True,
                                    tile_position=(0, 64))
                                nc.scalar.activation(
                                    qk_s[64:128, qi, csl, 0:8], psh[64:128, :],
                                    AF.Identity, scale=1.0)
                else:
                    nc.scalar.activation(
                        vt_s[0:64, qi, j, :], pj[0:64, :],
                        AF.Identity, bias=b_s[0:64, j:j + 1], scale=1.0)
                    nc.scalar.activation(
                        osv_s[64:128, qi, j, :], pj[64:128, :],
                        AF.Identity, bias=b_s[64:128, j:j + 1], scale=1.0)

            def shuffles(qq):
                qi = qq % 2
                nc.sync.dma_start(
                    out=vt_s[0:64, qi, 8:16, :], in_=osv_s[64:128, qi, :, :])
                nc.sync.dma_start(
                    out=vt_s[64:128, qi, :, :], in_=vt_s[0:64, qi, :, :])

            # ---- attention batch units (depth-2 pipeline: front(b); back(b-1))
            def attn_front(qq, b):
                qi = qq % 2
                pk_ps = psa.tile([128, 1024], b16, tag="ps", name="pk")
                for gi in range(8):
                    g = 8 * b + gi
                    nc.tensor.transpose(
                        pk_ps[:, 128 * gi:128 * gi + 128],
                        qk_s[:, qi, 4 * g:4 * g + 4, :], ident[:])
                pk = work.tile([128, 1024], b16, tag="pk")
                nc.vector.tensor_copy(pk[:], pk_ps[:])
                et_b = [psa.tile([128, 512], f32, tag="ps", name="et0"),
                        psa.tile([128, 512], f32, tag="ps", name="et1")]
                for gi in range(8):
                    off = 128 * gi
                    for tau in range(4):
                        nc.tensor.matmul(
                            et_b[tau // 2][64 * (tau % 2):64 * (tau % 2) + 64,
                                           64 * gi:64 * gi + 64],
                            pk[32 * tau:32 * tau + 32, off + 64:off + 128],
                            pk[32 * tau:32 * tau + 32, off:off + 64],
                            start=True, stop=True,
                            tile_position=(32 * tau, 64 * (tau % 2)))
                e_b = [epool.tile([128, 512], b16, tag="e0", name="e0"),
                       epool.tile([128, 512], b16, tag="e1", name="e1")]
                nc.scalar.activation(e_b[0][:], et_b[0][:], AF.Exp)
                nc.scalar.activation(e_b[1][:], et_b[1][:], AF.Exp)
                return e_b

            def attn_back(qq, b, e_b):
                qi = qq % 2
                qp, qoff = qq // 2, (qq % 2) * 128
                pa_b = psa.tile([128, 2, 8, 17], f32, tag="ps", name="pa")
                for gi in range(8):
                    for tau in range(4):
                        t = 32 * b + 4 * gi + tau
                        par = tau % 2
                        nc.tensor.matmul(
                            pa_b[64 * par:64 * par + 64, tau // 2, gi, :],
                            e_b[tau // 2][64 * par:64 * par + 64,
                                          64 * gi:64 * gi + 64],
                            vt_s[64 * par:64 * par + 64, qi, :, t],
                            start=True, stop=True)
                # tp = 16b + 2gi + x -> even/odd interleave per half
                for x in range(2):
                    zr = zpool.tile([128, 8], f32, tag="zr")
                    nc.vector.reciprocal(zr[:], pa_b[:, x, :, 16])
                    nc.vector.tensor_mul(
                        a_nm[:, qp, qoff + 16 * b + x:qoff + 16 * b + 16:2, :],
                        pa_b[:, x, :, 0:16],
                        zr[:, :, None].to_broadcast((128, 8, 16)))

            def oproj_unit(qp, mo):
                po = [psp.tile([128, 512], f32, tag="ps", name="po0"),
                      psp.tile([128, 512], f32, tag="ps", name="po1")]
                for mi in range(2):
                    m = 2 * mo + mi
                    for h in range(16):
                        for par in range(2):
                            nc.tensor.matmul(
                                po[par][:, 256 * mi:256 * mi + 256],
                                wo_s[64 * par:64 * par + 64, h,
                                     m * 128:(m + 1) * 128],
                                a_nm[64 * par:64 * par + 64, qp, :, h],
                                start=(h == 0), stop=(h == 15))
                for par in range(2):
                    for mi in range(2):
                        m = 2 * mo + mi
                        o_sb = outp.tile([128, TP2], f32, tag="o")
                        nc.scalar.activation(
                            o_sb[:], po[par][:, 256 * mi:256 * mi + 256],
                            AF.Identity, bias=bo_s[:, m:m + 1], scale=1.0)
                        nc.scalar.dma_start(out=out_d[qp, par, m, :, :], in_=o_sb[:])

            # ---- the interleaved schedule ----
            # chains of quarter qq run with attention batches of quarter qq-1
            CHAIN_ORDER = [("q", j) for j in range(8)] + \
                          [("k", j) for j in range(8)] + \
                          [("v", j) for j in range(8)]

            def quarter(qq, x_t, x_next):
                """Emit proj(qq) chains interleaved with attn(qq-1) batches
                (and oproj(0) units when qq == 3)."""
                pend = None  # (b, e_b) awaiting back-half
                for step in range(8):
                    for c in range(3):
                        kind, j = CHAIN_ORDER[3 * step + c]
                        chain(qq, kind, j, x_t)
                        if qq >= 1 and c == 0:
                            if pend is not None:
                                attn_back(qq - 1, pend[0], pend[1])
                            pend = (step, attn_front(qq - 1, step))
                if pend is not None:
                    attn_back(qq - 1, pend[0], pend[1])
                shuffles(qq)
                return x_next

            x_next = load_x(1)
            x_cur = quarter(0, x_cur, x_next)
            x_next = load_x(2)
            x_cur = quarter(1, x_cur, x_next)
            x_next = load_x(3)
            x_cur = quarter(2, x_cur, x_next)
            x_cur = quarter(3, x_cur, None)
            # epilogue: attn(3) depth-2 pipelined with oproj(0) units woven
            # in (keeps the PE dense / HAM warm), then oproj(pair1)
            pend = None
            for b in range(NB):
                if pend is not None:
                    attn_back(3, pend[0], pend[1])
                pend = (b, attn_front(3, b))
                if b % 2 == 1:
                    oproj_unit(0, b // 2)
            attn_back(3, pend[0], pend[1])
            for mo in range(4):
                oproj_unit(1, mo)
    nc.compile()
    return nc


def host_prep(q, k, v, w_q, b_q, w_k, b_k, w_v, b_v, w_o, b_o):
    j = np.arange(8)[:, None, None]
    hb = np.arange(2)[None, :, None]
    d = np.arange(64)[None, None, :]
    perm = (d * 16 + 2 * j + hb).reshape(-1)

    def prep_w(w, scale=1.0):
        wt = (w[perm, :].T.astype(np.float32) * scale).astype(bf16)
        return np.ascontiguousarray(wt.reshape(8, 128, 1024))

    com = dict(
        wq=prep_w(w_q, 0.125), wk=prep_w(w_k), wv=prep_w(w_v),
        bq=np.ascontiguousarray((b_q[perm] * 0.125).reshape(8, 128)).astype(np.float32),
        bk=np.ascontiguousarray(b_k[perm].reshape(8, 128)).astype(np.float32),
        bv=np.ascontiguousarray(b_v[perm].reshape(8, 128)).astype(np.float32),
        bo=np.ascontiguousarray(b_o.reshape(8, 128)).astype(np.float32),
    )
    # V slot order: slot j = h 2j (j<8), slot 8+j = h 2j+1
    hmap = np.array([2 * j for j in range(8)] + [2 * j + 1 for j in range(8)])
    wo_half = np.transpose(w_o.reshape(1024, 64, 16), (1, 2, 0))[:, hmap, :]
    com["wo"] = np.ascontiguousarray(
        np.concatenate([wo_half, wo_half], axis=0).astype(bf16))

    in_maps = []
    for c in range(NCORE):
        m = dict(com)
        for name, x in (("xq", q), ("xk", k), ("xv", v)):
            sl = x.reshape(-1, D)[c * T:(c + 1) * T, :]
            m[name] = np.ascontiguousarray(sl.T.astype(bf16).reshape(8, 128, T))
        in_maps.append(m)
    return in_maps


def reassemble(results):
    # per-core out [2 qp, 2 par, 8 m, 128 dm, 256 tp']; tp' = (qh, tp)
    # token t = qp*512 + qh*256 + 2*tp + par ; D = m*128 + dm
    full = np.empty((NCORE, T, D), np.float32)
    for c, res in enumerate(results):
        od = res["out"].reshape(2, 2, 8, 128, 2, 128)  # qp par m dm qh tp
        o = np.transpose(od, (0, 4, 5, 1, 2, 3))       # qp qh tp par m dm
        full[c] = o.reshape(T, D)
    return full.reshape(B, S, D)


def kernel(**inputs):
    from concourse.bass_utils import run_bass_kernel_spmd
    if "nc" not in _NC_CACHE:
        _NC_CACHE["nc"] = build_nc()
    nc = _NC_CACHE["nc"]
    in_maps = host_prep(**inputs)
    r = run_bass_kernel_spmd(nc, in_maps, core_ids=list(range(NCORE)))
    return reassemble(r.results)


if __name__ == "__main__":
    z = np.load("/root/problem/inputs_cache.npz")
    inputs = {kk: z[kk] for kk in z.files}
    expd = np.load("/root/problem/expected64.npy")
    act = kernel(**inputs)
    err = np.abs(act - expd)
    scale = np.abs(expd).max()
    print("absmax err:", err.max(), "rel:", err.max() / scale)


# revision 17
# speedup vs baseline: 1.3407x; 1.3407x over previous
"""Trainium2 Bass kernel for nn_MultiHeadAttention_59158879535767 (v3).

Reference semantics (B=4, S=2048, D=1024, H=16, DK=64):
  Q = q @ w_q.T + b_q  (same for K, V), reshaped (B,S,DK,H);
  score contracts over the HEAD axis per token: score[t] = Q_t @ K_t.T / 8
  (64x64 per token), softmax over last axis, attn[t] = score @ V_t -> (64,16),
  flattened, then @ w_o.T + b_o.

Everything is per-token => data-parallel over the 8192 tokens across 8 cores.
Per core T=1024 tokens in NQ=4 quarters of T4=256, in 32-token batches.

v3 (from v2b @ 470us, baseline 698us): one fully interleaved emission stream.
The PE must never idle >~3us or the HAM clock gate drops it from 2.4 to
1.2 GHz (measured: whole oproj phases ran cold).  Per quarter we emit
[3 proj j-chains + 1 attention batch] x 8 so that:
  * the PE queue always has dense matmul work while ACT runs exp and the
    strided evacs trail on DVE/ACT without blocking consumers queued behind
    a wholesale evac block (queues are strict FIFO per engine);
  * attention batches are depth-2 software-pipelined (S2(b-1) after S1(b))
    so the exp latency hides under the next batch's transposes;
  * oproj(pair0) units interleave into quarter 3, oproj(pair1) drains at
    the end (dense, so it stays warm).
Engine split: DVE = Q evacs (strided) + pack copies + normalize;
ACT = K/V evacs + exp + oproj evac.  Fused Q|K pack -> one [128,128]
transpose per 4 tokens; normalize straight from PSUM; initial weight DMAs
spread across engine queues with x(0) first.
"""
import numpy as np
import ml_dtypes

B, S, D, H, DK = 4, 2048, 1024, 16, 64
NCORE = 8
T = (B * S) // NCORE          # 1024 tokens per core
NQ = 4
T4 = T // NQ                  # 256 tokens per quarter
NB = T4 // 32                 # 8 batches of 32 tokens per quarter
TP2 = 256                     # tokens per parity per 2-quarter pair

bf16 = ml_dtypes.bfloat16

_NC_CACHE = {}


def build_nc():
    import concourse.bacc as bacc
    import concourse.mybir as mybir
    import concourse.tile as tile
    from concourse.masks import make_identity

    nc = bacc.Bacc()
    dt = mybir.dt
    f32, b16 = dt.float32, dt.bfloat16
    AF = mybir.ActivationFunctionType

    # ---- DRAM I/O ----
    xq_d = nc.dram_tensor("xq", [8, 128, T], b16, kind="ExternalInput")
    xk_d = nc.dram_tensor("xk", [8, 128, T], b16, kind="ExternalInput")
    xv_d = nc.dram_tensor("xv", [8, 128, T], b16, kind="ExternalInput")
    wq_d = nc.dram_tensor("wq", [8, 128, 1024], b16, kind="ExternalInput")
    wk_d = nc.dram_tensor("wk", [8, 128, 1024], b16, kind="ExternalInput")
    wv_d = nc.dram_tensor("wv", [8, 128, 1024], b16, kind="ExternalInput")
    wo_d = nc.dram_tensor("wo", [128, 16, 1024], b16, kind="ExternalInput")
    bq_d = nc.dram_tensor("bq", [8, 128], f32, kind="ExternalInput")
    bk_d = nc.dram_tensor("bk", [8, 128], f32, kind="ExternalInput")
    bv_d = nc.dram_tensor("bv", [8, 128], f32, kind="ExternalInput")
    bo_d = nc.dram_tensor("bo", [8, 128], f32, kind="ExternalInput")
    # out token map: t = qp*512 + qh*256 + 2*tp + par, D = m*128 + dm
    out_d = nc.dram_tensor("out", [2, 2, 8, 128, TP2], f32, kind="ExternalOutput")

    with tile.TileContext(nc) as tc:
        with (
            tc.tile_pool(name="const", bufs=1) as const,
            tc.tile_pool(name="xin", bufs=2) as xin,
            tc.tile_pool(name="work", bufs=2) as work,
            tc.tile_pool(name="qf", bufs=1) as qfp,
            tc.tile_pool(name="epool", bufs=2) as epool,
            tc.tile_pool(name="zpool", bufs=4) as zpool,
            tc.tile_pool(name="outp", bufs=4) as outp,
            tc.tile_pool(name="psp", bufs=4, space="PSUM") as psp,
            tc.tile_pool(name="psa", bufs=4, space="PSUM") as psa,
        ):
            # ---- persistent SBUF ----
            wq_s = const.tile([128, 8, 1024], b16, tag="wq")
            wk_s = const.tile([128, 8, 1024], b16, tag="wk")
            wv_s = const.tile([128, 8, 1024], b16, tag="wv")
            wo_s = const.tile([128, 16, 1024], b16, tag="wo")
            bq_s = const.tile([128, 8], f32, tag="bq")
            bk_s = const.tile([128, 8], f32, tag="bk")
            bv_s = const.tile([128, 8], f32, tag="bv")
            bo_s = const.tile([128, 8], f32, tag="bo")
            ident = const.tile([128, 128], b16, tag="ident")

            # per-quarter staging (x2: attn(q-1) overlaps proj(q))
            # qk: rows 0:64 Q^T [d, t, s], rows 64:128 K^T [e, t, s]
            qk_s = const.tile([128, 2, T4, 32], b16, tag="qk")
            # stg: rows 0:64 = K even-h (-> up-shift), 64:128 = Q odd-h (-> down)
            stg_s = const.tile([128, 2, T4, 8], b16, tag="stg")
            vt_s = const.tile([128, 2, 17, T4], b16, tag="vt")
            osv_s = const.tile([128, 2, 8, T4], b16, tag="osv")
            a_nm = const.tile([128, 2, 16, TP2], b16, tag="anorm")

            # x(0) first so proj(0) can start ASAP; weights spread across
            # engine DMA queues so they load in parallel with x and each other
            def load_x(qq):
                tsl = slice(qq * T4, (qq + 1) * T4)
                xt = []
                for nm, xd in (("xq", xq_d), ("xk", xk_d), ("xv", xv_d)):
                    t_ = xin.tile([128, 8, T4], b16, tag=nm)
                    nc.sync.dma_start(out=t_[:], in_=xd[:, :, tsl].rearrange("ko p t -> p ko t"))
                    xt.append(t_)
                return xt

            x_cur = load_x(0)
            for bd, bsb in ((bq_d, bq_s), (bk_d, bk_s), (bv_d, bv_s), (bo_d, bo_s)):
                nc.sync.dma_start(out=bsb[:], in_=bd.rearrange("j p -> p j"))
            # weight loads split across both hwdge queues, x(0) already ahead
            nc.scalar.dma_start(out=wq_s[:], in_=wq_d.rearrange("ko p m -> p ko m"))
            nc.scalar.dma_start(out=wk_s[:], in_=wk_d.rearrange("ko p m -> p ko m"))
            nc.scalar.dma_start(out=wv_s[:], in_=wv_d.rearrange("ko p m -> p ko m"))
            nc.sync.dma_start(out=wo_s[:], in_=wo_d[:])
            make_identity(nc, ident)
            nc.any.memset(qk_s[:, :, :, 16:32], 0.0)
            nc.any.memset(vt_s[0:64, :, 16, :], 1.0)

            W = {"q": (wq_s, bq_s), "k": (wk_s, bk_s), "v": (wv_s, bv_s)}

            # per-quarter contiguous projection staging (pre-scatter)
            qf_cur = {}

            def chain(qq, kind, j, x_t):
                """One projection j-chain: 8 accumulating MMs + evac.

                PSUM evac is ALWAYS contiguous on ACT (357ns measured; a
                strided ACT/DVE evac costs 1.1-1.6us).  Q/K land in a flat
                [128, 8, T4] staging tile; one bulk DVE SBUF->SBUF scatter
                per projection half then builds the t-major pack layout
                (strided SBUF->SBUF on DVE is cheap)."""
                qi = qq % 2
                x = x_t["qkv".index(kind)]
                w_s, b_s = W[kind]
                pj = psp.tile([128, 512], f32, tag="ps", name="pj")[:, :T4]
                for ko in range(8):
                    nc.tensor.matmul(
                        pj, w_s[:, ko, j * 128:(j + 1) * 128], x[:, ko, :],
                        start=(ko == 0), stop=(ko == 7))
                if kind in ("q", "k"):
                    if j == 0:
                        qf_cur[kind] = qfp.tile(
                            [128, 8, T4], b16, tag="qf" + kind, name="qf" + kind)
                    qf = qf_cur[kind]
                    nc.scalar.activation(
                        qf[:, j, :], pj[:, :],
                        AF.Identity, bias=b_s[:, j:j + 1], scale=1.0)
                    if j == 7:
                        src_lo = qf[0:64, :, :].rearrange("p j t -> p t j")
                        src_hi = qf[64:128, :, :].rearrange("p j t -> p t j")
                        if kind == "q":
                            # even h -> direct slots 0:8; odd h -> stg (down)
                            nc.vector.tensor_copy(qk_s[0:64, qi, :, 0:8], src_lo)
                            nc.vector.tensor_copy(stg_s[64:128, qi, :, :], src_hi)
                            for cc in range(4):
                                csl = slice(64 * cc, 64 * cc + 64)
                                psh = psp.tile([128, 512], f32, tag="ps",
                                               name="pshq")
                                nc.tensor.matmul(
                                    psh[0:64, :], ident[64:128, 64:128],
                                    stg_s[64:128, qi, csl, :],
                                    start=True, stop=True,
                                    tile_position=(64, 0))
                                nc.scalar.activation(
                                    qk_s[0:64, qi, csl, 8:16], psh[0:64, :],
                                    AF.Identity, scale=1.0)
                        else:
                            # even h -> stg (up); odd h -> direct slots 8:16
                            nc.vector.tensor_copy(stg_s[0:64, qi, :, :], src_lo)
                            nc.vector.tensor_copy(qk_s[64:128, qi, :, 8:16], src_hi)
                            for cc in range(4):
                                csl = slice(64 * cc, 64 * cc + 64)
                                psh = psp.tile([128, 512], f32, tag="ps",
                                               name="pshk")
                                nc.tensor.matmul(
                                    psh[64:128, :], ident[0:64, 0:64],
                                    stg_s[0:64, qi, csl, :],
                                    start=True, stop=True,
                                    tile_position=(0, 64))
                                nc.scalar.activation(
                                    qk_s[64:128, qi, csl, 0:8], psh[64:128, :],
                                    AF.Identity, scale=1.0)
                else:
                    nc.scalar.activation(
                        vt_s[0:64, qi, j, :], pj[0:64, :],
                        AF.Identity, bias=b_s[0:64, j:j + 1], scale=1.0)
                    nc.scalar.activation(
                        osv_s[64:128, qi, j, :], pj[64:128, :],
                        AF.Identity, bias=b_s[64:128, j:j + 1], scale=1.0)

            def shuffles(qq):
                qi = qq % 2
                nc.sync.dma_start(
                    out=vt_s[0:64, qi, 8:16, :], in_=osv_s[64:128, qi, :, :])
                nc.sync.dma_start(
                    out=vt_s[64:128, qi, :, :], in_=vt_s[0:64, qi, :, :])

            # ---- attention batch units (depth-2 pipeline: front(b); back(b-1))
            def attn_front(qq, b):
                qi = qq % 2
                pk_ps = psa.tile([128, 1024], b16, tag="ps", name="pk")
                for gi in range(8):
                    g = 8 * b + gi
                    nc.tensor.transpose(
                        pk_ps[:, 128 * gi:128 * gi + 128],
                        qk_s[:, qi, 4 * g:4 * g + 4, :], ident[:])
                pk = work.tile([128, 1024], b16, tag="pk")
                nc.vector.tensor_copy(pk[:], pk_ps[:])
                et_b = [psa.tile([128, 512], f32, tag="ps", name="et0"),
                        psa.tile([128, 512], f32, tag="ps", name="et1")]
                for gi in range(8):
                    off = 128 * gi
                    for tau in range(4):
                        nc.tensor.matmul(
                            et_b[tau // 2][64 * (tau % 2):64 * (tau % 2) + 64,
                                           64 * gi:64 * gi + 64],
                            pk[32 * tau:32 * tau + 32, off + 64:off + 128],
                            pk[32 * tau:32 * tau + 32, off:off + 64],
                            start=True, stop=True,
                            tile_position=(32 * tau, 64 * (tau % 2)))
                e_b = [epool.tile([128, 512], b16, tag="e0", name="e0"),
                       epool.tile([128, 512], b16, tag="e1", name="e1")]
                nc.scalar.activation(e_b[0][:], et_b[0][:], AF.Exp)
                nc.scalar.activation(e_b[1][:], et_b[1][:], AF.Exp)
                return e_b

            def attn_back(qq, b, e_b):
                qi = qq % 2
                qp, qoff = qq // 2, (qq % 2) * 128
                pa_b = psa.tile([128, 2, 8, 17], f32, tag="ps", name="pa")
                for gi in range(8):
                    for tau in range(4):
                        t = 32 * b + 4 * gi + tau
                        par = tau % 2
                        nc.tensor.matmul(
                            pa_b[64 * par:64 * par + 64, tau // 2, gi, :],
                            e_b[tau // 2][64 * par:64 * par + 64,
                                          64 * gi:64 * gi + 64],
                            vt_s[64 * par:64 * par + 64, qi, :, t],
                            start=True, stop=True)
                # tp = 16b + 2gi + x -> even/odd interleave per half
                for x in range(2):
                    zr = zpool.tile([128, 8], f32, tag="zr")
                    nc.vector.reciprocal(zr[:], pa_b[:, x, :, 16])
                    nc.vector.tensor_mul(
                        a_nm[:, qp, :, qoff + 16 * b + x:qoff + 16 * b + 16:2]
                        .rearrange("p h t -> p t h"),
                        pa_b[:, x, :, 0:16],
                        zr[:, :, None].to_broadcast((128, 8, 16)))

            def oproj_unit(qp, mo):
                po = [psp.tile([128, 512], f32, tag="ps", name="po0"),
                      psp.tile([128, 512], f32, tag="ps", name="po1")]
                for mi in range(2):
                    m = 2 * mo + mi
                    for h in range(16):
                        for par in range(2):
                            nc.tensor.matmul(
                                po[par][:, 256 * mi:256 * mi + 256],
                                wo_s[64 * par:64 * par + 64, h,
                                     m * 128:(m + 1) * 128],
                                a_nm[64 * par:64 * par + 64, qp, h, :],
                                start=(h == 0), stop=(h == 15))
                for par in range(2):
                    for mi in range(2):
                        m = 2 * mo + mi
                        o_sb = outp.tile([128, TP2], f32, tag="o")
                        nc.scalar.activation(
                            o_sb[:], po[par][:, 256 * mi:256 * mi + 256],
                            AF.Identity, bias=bo_s[:, m:m + 1], scale=1.0)
                        nc.scalar.dma_start(out=out_d[qp, par, m, :, :], in_=o_sb[:])

            # ---- the interleaved schedule ----
            # chains of quarter qq run with attention batches of quarter qq-1
            CHAIN_ORDER = [("q", j) for j in range(8)] + \
                          [("k", j) for j in range(8)] + \
                          [("v", j) for j in range(8)]

            def quarter(qq, x_t, x_next):
                """Emit proj(qq) chains interleaved with attn(qq-1) batches
                (and oproj(0) units when qq == 3)."""
                pend = None  # (b, e_b) awaiting back-half
                for step in range(8):
                    for c in range(3):
                        kind, j = CHAIN_ORDER[3 * step + c]
                        chain(qq, kind, j, x_t)
                        if qq >= 1 and c == 0:
                            if pend is not None:
                                attn_back(qq - 1, pend[0], pend[1])
                            pend = (step, attn_front(qq - 1, step))
                if pend is not None:
                    attn_back(qq - 1, pend[0], pend[1])
                shuffles(qq)
                return x_next

            x_next = load_x(1)
            x_cur = quarter(0, x_cur, x_next)
            x_next = load_x(2)
            x_cur = quarter(1, x_cur, x_next)
            x_next = load_x(3)
            x_cur = quarter(2, x_cur, x_next)
            x_cur = quarter(3, x_cur, None)
            # epilogue: attn(3) depth-2 pipelined with oproj(0) units woven
            # in (keeps the PE dense / HAM warm), then oproj(pair1)
            pend = None
            for b in range(NB):
                if pend is not None:
                    attn_back(3, pend[0], pend[1])
                pend = (b, attn_front(3, b))
                if b % 2 == 1:
                    oproj_unit(0, b // 2)
            attn_back(3, pend[0], pend[1])
            for mo in range(4):
                oproj_unit(1, mo)
    nc.compile()
    return nc


def host_prep(q, k, v, w_q, b_q, w_k, b_k, w_v, b_v, w_o, b_o):
    j = np.arange(8)[:, None, None]
    hb = np.arange(2)[None, :, None]
    d = np.arange(64)[None, None, :]
    perm = (d * 16 + 2 * j + hb).reshape(-1)

    def prep_w(w, scale=1.0):
        wt = (w[perm, :].T.astype(np.float32) * scale).astype(bf16)
        return np.ascontiguousarray(wt.reshape(8, 128, 1024))

    com = dict(
        wq=prep_w(w_q, 0.125), wk=prep_w(w_k), wv=prep_w(w_v),
        bq=np.ascontiguousarray((b_q[perm] * 0.125).reshape(8, 128)).astype(np.float32),
        bk=np.ascontiguousarray(b_k[perm].reshape(8, 128)).astype(np.float32),
        bv=np.ascontiguousarray(b_v[perm].reshape(8, 128)).astype(np.float32),
        bo=np.ascontiguousarray(b_o.reshape(8, 128)).astype(np.float32),
    )
    # V slot order: slot j = h 2j (j<8), slot 8+j = h 2j+1
    hmap = np.array([2 * j for j in range(8)] + [2 * j + 1 for j in range(8)])
    wo_half = np.transpose(w_o.reshape(1024, 64, 16), (1, 2, 0))[:, hmap, :]
    com["wo"] = np.ascontiguousarray(
        np.concatenate([wo_half, wo_half], axis=0).astype(bf16))

    in_maps = []
    for c in range(NCORE):
        m = dict(com)
        for name, x in (("xq", q), ("xk", k), ("xv", v)):
            sl = x.reshape(-1, D)[c * T:(c + 1) * T, :]
            m[name] = np.ascontiguousarray(sl.T.astype(bf16).reshape(8, 128, T))
        in_maps.append(m)
    return in_maps


def reassemble(results):
    # per-core out [2 qp, 2 par, 8 m, 128 dm, 256 tp']; tp' = (qh, tp)
    # token t = qp*512 + qh*256 + 2*tp + par ; D = m*128 + dm
    full = np.empty((NCORE, T, D), np.float32)
    for c, res in enumerate(results):
        od = res["out"].reshape(2, 2, 8, 128, 2, 128)  # qp par m dm qh tp
        o = np.transpose(od, (0, 4, 5, 1, 2, 3))       # qp qh tp par m dm
        full[c] = o.reshape(T, D)
    return full.reshape(B, S, D)


def kernel(**inputs):
    from concourse.bass_utils import run_bass_kernel_spmd
    if "nc" not in _NC_CACHE:
        _NC_CACHE["nc"] = build_nc()
    nc = _NC_CACHE["nc"]
    in_maps = host_prep(**inputs)
    r = run_bass_kernel_spmd(nc, in_maps, core_ids=list(range(NCORE)))
    return reassemble(r.results)


if __name__ == "__main__":
    z = np.load("/root/problem/inputs_cache.npz")
    inputs = {kk: z[kk] for kk in z.files}
    expd = np.load("/root/problem/expected64.npy")
    act = kernel(**inputs)
    err = np.abs(act - expd)
    scale = np.abs(expd).max()
    print("absmax err:", err.max(), "rel:", err.max() / scale)
